# revision 7
# baseline (speedup 1.0000x reference)
"""Multi-head attention (softmax_one variant) on 8 Trainium2 NeuronCores.

Sharding: core c handles batch b = c // 4 and head group g = c % 4
(heads 4g..4g+3, 256 projected dims). Inside each core everything is
computed in "transposed activation" layout [feature, token] so that the
token dim is always the matmul moving dim:

  qT/kT = W^T-parallel projections           [256, 2048]
  v     = natural layout                     [2048, 256]
  pass1 = S[i,j] tiles -> exp/accum stats -> r_i = 1/(max E + sum E)
  pass2 = S^T[j,i] tiles -> E^T -> O^T accum [64, 2048] per head
  normalization: O^T * r (r broadcast across partitions via K=1 matmul)
  out-proj partial P = O^T.T @ WoT + bo/4    [2048, 1024]

Host reduces the 4 partial P per batch (row-parallel out projection) and
reassembles present = stack(k, v).
"""

from contextlib import ExitStack

import numpy as np

import concourse.bass as bass
import concourse.mybir as mybir
import concourse.tile as tile
from concourse import bacc
from concourse.bass_utils import run_bass_kernel_spmd
from concourse.masks import make_identity

F32 = mybir.dt.float32
F32R = mybir.dt.float32r
AX = mybir.AxisListType.X
EXP = mybir.ActivationFunctionType.Exp

B = 2
N = 2048
DM = 1024
H = 16
DH = 64
NCORES = 8
GROUPS = 4          # head groups (cores) per batch
HPC = H // GROUPS   # heads per core = 4
DC = HPC * DH       # projected dims per core = 256
P = 128
NT = N // P         # 16 token tiles
NJ = N // 512       # 4 token chunks of 512
KO = DM // P        # 8 contraction subtiles for projections
SCALE = 1.0 / 8.0   # 1/sqrt(DH)





def build_kernel():
    nc = bacc.Bacc("TRN2", target_bir_lowering=False, debug=False,
                   num_devices=NCORES)

    xq = nc.dram_tensor("xq_t", [DM, N], F32R, kind="ExternalInput").ap()
    xk = nc.dram_tensor("xk_t", [DM, N], F32R, kind="ExternalInput").ap()
    xv = nc.dram_tensor("xv_t", [DM, N], F32R, kind="ExternalInput").ap()
    wq = nc.dram_tensor("wq_t", [DM, DC], F32R, kind="ExternalInput").ap()
    wk = nc.dram_tensor("wk_t", [DM, DC], F32R, kind="ExternalInput").ap()
    wv = nc.dram_tensor("wv_t", [DM, DC], F32R, kind="ExternalInput").ap()
    wo = nc.dram_tensor("wo_t", [DC, DM], F32R, kind="ExternalInput").ap()
    bq = nc.dram_tensor("bq_c", [1, DC], F32R, kind="ExternalInput").ap()
    bk = nc.dram_tensor("bk_c", [1, DC], F32R, kind="ExternalInput").ap()
    bv = nc.dram_tensor("bv_c", [1, DC], F32R, kind="ExternalInput").ap()
    bo = nc.dram_tensor("bo_q", [1, DM], F32R, kind="ExternalInput").ap()
    onz = nc.dram_tensor("ones_c", [1, 512], F32R, kind="ExternalInput").ap()

    kt_out = nc.dram_tensor("kt_out", [DC, N], F32, kind="ExternalOutput").ap()
    v_out = nc.dram_tensor("v_out", [N, DC], F32, kind="ExternalOutput").ap()
    p_out = nc.dram_tensor("p_out", [N, DM], F32, kind="ExternalOutput").ap()

    with tile.TileContext(nc) as tc:
        with ExitStack() as ctx:
            _body(ctx, tc, xq, xk, xv, wq, wk, wv, wo, bq, bk, bv, bo, onz,
                  kt_out, v_out, p_out)
    nc.compile()
    return nc


def _body(ctx, tc, xq, xk, xv, wq, wk, wv, wo, bq, bk, bv, bo, onz,
          kt_out, v_out, p_out):
    nc = tc.nc

    cpool = ctx.enter_context(tc.tile_pool(name="const", bufs=1))
    wpool = ctx.enter_context(tc.tile_pool(name="weights", bufs=1))
    apool = ctx.enter_context(tc.tile_pool(name="acts", bufs=1))
    xpool = ctx.enter_context(tc.tile_pool(name="xstream", bufs=3))
    epool = ctx.enter_context(tc.tile_pool(name="etiles", bufs=4))
    spool = ctx.enter_context(tc.tile_pool(name="stats", bufs=4))
    rpool = ctx.enter_context(tc.tile_pool(name="rload", bufs=2))
    ppool = ctx.enter_context(tc.tile_pool(name="pout", bufs=3))
    dpool = ctx.enter_context(tc.tile_pool(name="dram", bufs=2, space="DRAM"))
    ps_mm = ctx.enter_context(tc.tile_pool(name="ps_mm", bufs=3, space="PSUM"))
    ps_av = ctx.enter_context(tc.tile_pool(name="ps_av", bufs=1, space="PSUM"))
    ps_tr = ctx.enter_context(tc.tile_pool(name="ps_tr", bufs=1, space="PSUM"))

    # --- constants -------------------------------------------------------
    ident = cpool.tile([P, P], F32)
    make_identity(nc, ident)
    ones = cpool.tile([1, 512], F32R)
    nc.sync.dma_start(ones[:], onz)

    bq_sb = cpool.tile([1, DC], F32R, tag="bq")
    bk_sb = cpool.tile([1, DC], F32R, tag="bk")
    bv_sb = cpool.tile([1, DC], F32R, tag="bv")
    bo_sb = cpool.tile([1, DM], F32R, tag="bo")
    nc.sync.dma_start(bq_sb[:], bq)
    nc.sync.dma_start(bk_sb[:], bk)
    nc.sync.dma_start(bv_sb[:], bv)
    nc.sync.dma_start(bo_sb[:], bo)

    # --- weights ---------------------------------------------------------
    wq_sb = wpool.tile([P, KO, DC], F32R, tag="wq")
    wk_sb = wpool.tile([P, KO, DC], F32R, tag="wk")
    wv_sb = wpool.tile([P, KO, DC], F32R, tag="wv")
    wo_sb = wpool.tile([P, DC // P, DM], F32R, tag="wo")
    nc.sync.dma_start(wq_sb[:], wq.rearrange("(ko p) c -> p ko c", p=P))
    nc.sync.dma_start(wk_sb[:], wk.rearrange("(ko p) c -> p ko c", p=P))
    nc.sync.dma_start(wv_sb[:], wv.rearrange("(ko p) c -> p ko c", p=P))
    nc.sync.dma_start(wo_sb[:], wo.rearrange("(kt p) e -> p kt e", p=P))

    # --- persistent activations -----------------------------------------
    qT = apool.tile([P, DC // P, N], F32R, tag="qT")   # [128, 2, 2048]
    kT = apool.tile([P, DC // P, N], F32R, tag="kT")
    vN = apool.tile([P, NT, DC], F32R, tag="vN")       # [128, 16, 256]
    oT = apool.tile([P, DC // P, N], F32R, tag="oT")

    # === stage A: projections ===========================================
    for x_dram, w_sb, b_sb, outT in ((xq, wq_sb, bq_sb, qT),
                                     (xk, wk_sb, bk_sb, kT)):
        xr = x_dram.rearrange("(ko p) n -> p ko n", p=P)
        for jc in range(NJ):
            xt = xpool.tile([P, KO, 512], F32R, tag="xt")
            nc.sync.dma_start(xt[:], xr[:, :, jc * 512:(jc + 1) * 512])
            for m in range(DC // P):
                ps = ps_mm.tile([P, 512], F32, tag="s")
                for ko in range(KO):
                    nc.tensor.matmul(
                        ps[:], w_sb[:, ko, m * P:(m + 1) * P],
                        xt[:, ko, :], start=(ko == 0), stop=False)
                nc.tensor.matmul(ps[:], b_sb[:, m * P:(m + 1) * P],
                                 ones[:], start=False, stop=True)
                nc.scalar.copy(outT[:, m, jc * 512:(jc + 1) * 512], ps[:])

    nc.sync.dma_start(kt_out.rearrange("(m p) n -> p m n", p=P),
                      kT[:].bitcast(F32))

    xvr = xv.rearrange("(ko p) n -> p ko n", p=P)
    for jc in range(NJ):
        xt = xpool.tile([P, KO, 512], F32R, tag="xt")
        nc.sync.dma_start(xt[:], xvr[:, :, jc * 512:(jc + 1) * 512])
        for tt in range(4):
            nt = jc * 4 + tt
            ps = ps_mm.tile([P, 512], F32, tag="s")
            for ko in range(KO):
                nc.tensor.matmul(
                    ps[:, :DC], xt[:, ko, tt * P:(tt + 1) * P],
                    wv_sb[:, ko, :], start=(ko == 0), stop=False)
            nc.tensor.matmul(ps[:, :DC], ones[:, :P], bv_sb[:],
                             start=False, stop=True)
            nc.scalar.copy(vN[:, nt, :], ps[:, :DC])

    nc.sync.dma_start(v_out.rearrange("(nt p) c -> p nt c", p=P),
                      vN[:].bitcast(F32))

    # === stage B: attention per head ====================================
    for h in range(HPC):
        po = h % 2 * DH          # partition offset of this head in qT/kT/oT
        mt = h // 2              # which 128-row tile of qT/kT/oT
        qh = qT[po:po + DH, mt, :]
        kh = kT[po:po + DH, mt, :]

        # --- pass 1: row stats -----------------------------------------
        rinv = spool.tile([P, NT], F32, tag="rinv")
        for i0 in range(NT):
            sacc = spool.tile([P, NJ], F32, tag="sacc")
            emax = spool.tile([P, NJ], F32, tag="emax")
            for jc in range(NJ):
                ps = ps_mm.tile([P, 512], F32, tag="s")
                nc.tensor.matmul(ps[:], qh[:, i0 * P:(i0 + 1) * P],
                                 kh[:, jc * 512:(jc + 1) * 512],
                                 start=True, stop=True)
                esc = epool.tile([P, 512], F32, tag="esc")
                nc.scalar.activation(esc[:], ps[:], EXP, scale=SCALE,
                                     accum_out=sacc[:, jc:jc + 1])
                nc.vector.reduce_max(emax[:, jc:jc + 1], esc[:], axis=AX)
            den = spool.tile([P, 2], F32, tag="den")
            nc.vector.reduce_sum(den[:, 0:1], sacc[:], axis=AX)
            nc.vector.reduce_max(den[:, 1:2], emax[:], axis=AX)
            dsum = spool.tile([P, 1], F32, tag="dsum")
            nc.vector.tensor_add(dsum[:], den[:, 0:1], den[:, 1:2])
            nc.vector.reciprocal(rinv[:, i0:i0 + 1], dsum[:])

        # r [128, 16] -> row [1, 2048] (PE transpose + DRAM bounce)
        ps_t = ps_tr.tile([NT, P], F32, tag="tr")
        nc.tensor.transpose(ps_t[:], rinv[:], ident[:])
        rr16 = rpool.tile([NT, P], F32R, tag="rr16")
        nc.scalar.copy(rr16[:], ps_t[:])
        dscratch = dpool.tile([NT, P], F32R, tag="dscr")
        nc.sync.dma_start(dscratch[:], rr16[:])
        rrow = rpool.tile([1, N], F32R, tag="rrow")
        nc.sync.dma_start(rrow[:], dscratch.rearrange("t p -> (t p)")[None, :])

        # --- pass 2: S^T tiles, E^T, O^T accumulation -------------------
        pos = [ps_av.tile([DH, 512], F32, tag=f"po{ic}",
                           name=f"po{ic}") for ic in range(NJ)]
        for j0 in range(NT):
            vh = vN[:, j0, h * DH:(h + 1) * DH]
            for ic in range(NJ):
                ps = ps_mm.tile([P, 512], F32, tag="s")
                nc.tensor.matmul(ps[:], kh[:, j0 * P:(j0 + 1) * P],
                                 qh[:, ic * 512:(ic + 1) * 512],
                                 start=True, stop=True)
                et = epool.tile([P, 512], F32R, tag="et")
                nc.scalar.activation(et[:], ps[:], EXP, scale=SCALE)
                nc.tensor.matmul(pos[ic][:], vh, et[:],
                                 start=(j0 == 0), stop=(j0 == NT - 1))

        for ic in range(NJ):
            pr = ps_mm.tile([P, 512], F32, tag="s")
            nc.tensor.matmul(pr[:DH, :], ones[:, :DH],
                             rrow[:, ic * 512:(ic + 1) * 512],
                             start=True, stop=True)
            rb = epool.tile([DH, 512], F32, tag="rb")
            nc.scalar.copy(rb[:], pr[:DH, :])
            nc.vector.tensor_mul(oT[po:po + DH, mt, ic * 512:(ic + 1) * 512],
                                 pos[ic][:], rb[:])

    # === stage C: output projection (partial) ===========================
    bo_b = apool.tile([P, DM], F32, tag="bo_b")
    for ec in range(2):
        pb = ps_mm.tile([P, 512], F32, tag="s")
        nc.tensor.matmul(pb[:], ones[:, :P],
                         bo_sb[:, ec * 512:(ec + 1) * 512],
                         start=True, stop=True)
        nc.scalar.copy(bo_b[:, ec * 512:(ec + 1) * 512], pb[:])

    pr_out = p_out.rearrange("(nt p) e -> p nt e", p=P)
    for ntile in range(NT):
        for ec in range(2):
            ps = ps_mm.tile([P, 512], F32, tag="s")
            for kt2 in range(DC // P):
                nc.tensor.matmul(
                    ps[:], oT[:, kt2, ntile * P:(ntile + 1) * P],
                    wo_sb[:, kt2, ec * 512:(ec + 1) * 512],
                    start=(kt2 == 0), stop=(kt2 == DC // P - 1))
            pt = ppool.tile([P, 512], F32, tag="pt")
            nc.vector.tensor_add(pt[:], ps[:], bo_b[:, ec * 512:(ec + 1) * 512])
            nc.sync.dma_start(pr_out[:, ntile, ec * 512:(ec + 1) * 512], pt[:])


def shard_inputs(queries, keys, values, Wq, bq, Wk, bk, Wv, bv, Wo, bo):
    """Build the 8 per-core input maps (host-side layout prep)."""
    in_maps = []
    for c in range(NCORES):
        b = c // GROUPS
        g = c % GROUPS
        rows = slice(g * DC, (g + 1) * DC)
        m = {
            "xq_t": np.ascontiguousarray(queries[b].T),
            "xk_t": np.ascontiguousarray(keys[b].T),
            "xv_t": np.ascontiguousarray(values[b].T),
            "wq_t": np.ascontiguousarray(Wq[rows].T),
            "wk_t": np.ascontiguousarray(Wk[rows].T),
            "wv_t": np.ascontiguousarray(Wv[rows].T),
            "wo_t": np.ascontiguousarray(Wo[:, rows].T),
            "bq_c": np.ascontiguousarray(bq[rows])[None, :],
            "bk_c": np.ascontiguousarray(bk[rows])[None, :],
            "bv_c": np.ascontiguousarray(bv[rows])[None, :],
            "bo_q": (bo / GROUPS).astype(np.float32)[None, :],
            "ones_c": np.ones((1, 512), np.float32),
        }
        in_maps.append(m)
    return in_maps


def unshard_outputs(results):
    """results: list of 8 dicts with kt_out/v_out/p_out -> (out, present)."""
    out = np.zeros((B, N, DM), dtype=np.float32)
    present = np.zeros((2, B, H, N, DH), dtype=np.float32)
    for c in range(NCORES):
        b = c // GROUPS
        g = c % GROUPS
        r = results[c]
        out[b] += r["p_out"]
        kt = r["kt_out"].reshape(HPC, DH, N)          # [4, 64, 2048]
        present[0, b, g * HPC:(g + 1) * HPC] = kt.transpose(0, 2, 1)
        v = r["v_out"].reshape(N, HPC, DH)            # [2048, 4, 64]
        present[1, b, g * HPC:(g + 1) * HPC] = v.transpose(1, 0, 2)
    return out, present


_NC_CACHE = None


def get_nc():
    global _NC_CACHE
    if _NC_CACHE is None:
        _NC_CACHE = build_kernel()
    return _NC_CACHE


def kernel(queries, keys, values, Wq, bq, Wk, bk, Wv, bv, Wo, bo):
    nc = get_nc()
    in_maps = shard_inputs(queries, keys, values, Wq, bq, Wk, bk, Wv, bv,
                           Wo, bo)
    res = run_bass_kernel_spmd(nc, in_maps, list(range(NCORES)))
    return unshard_outputs(res.results)
